# revision 52
# baseline (speedup 1.0000x reference)
"""Deformable conv (nn_DeformConv_31267361915085) Trainium2 Bass kernel, v2.

Sharding: data-parallel over (batch, H-half): core n handles batch n//2,
output rows [28*(n%2), 28*(n%2)+28). Weights replicated. SPMD: one program;
per-core input slabs are pre-shifted on host so the program is core-agnostic.

v2 pipeline (per core, 13 blocks x 128 raster pixels):
  1. offset conv: 9 taps x 2 c-chunks of bf16 matmuls, PSUM-accumulated
     (PE pre-warmed with junk matmuls so it ramps to full clock).
  2. PE-transpose offsets to pixel-on-partition layout; coordinate math and
     bilinear corner weights (alpha) on DVE.
  3. per block: SWDGE dma_gather of 2x2 "quad" corner vectors (bf16, 2KB
     elements) from a zero-padded channels-last quad table in DRAM.
     Gathers round-robin over 4 SWDGE queues so Q7 descriptor generation
     runs on 4 core-pairs in parallel.
  4. bilinear lerp is fused into the (mandatory) patch transpose on the
     TENSOR engine: pt^T[c,p] = sum_q g_q^T @ diag(alpha_q).  diag(alpha)
     tiles are built on DVE as ident * alpha (one tensor_tensor per block).
  5. per block main conv: 36 bf16 matmuls, PSUM accumulate, DMA out.
"""

import sys

if "/opt/trn_rl_repo" not in sys.path:
    sys.path.insert(0, "/opt/trn_rl_repo")

import contextlib

import numpy as np
import ml_dtypes

import concourse.bass as bass
import concourse.tile as tile
from concourse import bacc, mybir
from concourse.bass_utils import run_bass_kernel_spmd
from concourse.masks import make_identity

F32 = mybir.dt.float32
BF16 = mybir.dt.bfloat16
F8E3 = mybir.dt.float8e3
I16 = mybir.dt.int16
I32 = mybir.dt.int32
AL = mybir.AluOpType

# problem dims
B, CIN, H, W = 4, 256, 56, 56
COUT = 256
KK = 9
MARG = 8                # gather pad margin (covers |offset| <= ~6)
HQ = WQ = H + 2 * MARG  # 72: quad-table grid
NQ = HQ * WQ            # 5184 quad rows
NROWS = 28              # output rows per core
NPIX = NROWS * W        # 1568
BLK = 128               # pixels per block (raster order)
NBLK = 13               # ceil(1568/128) = 12.25 -> 13 (last block 96 pad)
NSLOT = NBLK * BLK      # 1664
NIDX = KK * BLK         # 1152 gather indices per block

_CACHE = {}


def _ap(base, offset_elems, dims):
    """AP with explicit free dims on top of a tile's base AP."""
    return bass.AP(
        tensor=base.tensor, offset=base.offset + offset_elems, ap=[base.ap[0]] + dims
    )


def build_nc():
    nc = bacc.Bacc(None, target_bir_lowering=False, num_swdge_queues=4)

    xcf_d = nc.dram_tensor("xcf", [128, 2, 30 * 58], BF16, kind="ExternalInput")
    xq_d = nc.dram_tensor("xq", [NQ, 1024], F8E3, kind="ExternalInput")
    woff_d = nc.dram_tensor("woff", [128, 2, KK, 18], BF16, kind="ExternalInput")
    boff_d = nc.dram_tensor("boff", [18, 1], F32, kind="ExternalInput")
    wm_d = nc.dram_tensor("wm", [128, KK, 2, 2, 128], BF16, kind="ExternalInput")
    out_d = nc.dram_tensor("out", [128, 2, NSLOT], F32, kind="ExternalOutput")

    with tile.TileContext(nc) as tc, contextlib.ExitStack() as ctx:
        singles = ctx.enter_context(tc.tile_pool(name="singles", bufs=1))
        coords = ctx.enter_context(tc.tile_pool(name="coords", bufs=1))

        # ---- load constants / weights / activations ----
        xcf = singles.tile([128, 2, 30 * 58], BF16)
        nc.sync.dma_start(out=xcf[:, :, :], in_=xcf_d[:, :, :])
        woff = singles.tile([128, 2, KK, 18], BF16)
        nc.sync.dma_start(out=woff[:, :, :, :], in_=woff_d[:, :, :, :])
        boff = singles.tile([18, 1], F32)
        nc.sync.dma_start(out=boff[:, :], in_=boff_d[:, :])
        wm = singles.tile([128, KK, 2, 2, 128], BF16)
        nc.sync.dma_start(out=wm[:, :, :, :, :], in_=wm_d[:, :, :, :, :])

        ident_f = singles.tile([128, 128], F32)
        make_identity(nc, ident_f[:, :])
        ident_b = singles.tile([128, 128], BF16)
        nc.vector.tensor_copy(out=ident_b[:, :], in_=ident_f[:, :])

        # ---- PE warmup: ramp the clock while input DMAs land ----
        with tc.tile_pool(name="pwarm", bufs=1, space="PSUM") as pw:
            wps = pw.tile([128, 128], F32)
            for _ in range(8):
                nc.tensor.matmul(
                    wps[:, :], ident_f[:, :], ident_f[:, :], start=True, stop=True
                )

        # ---- offset conv: off_sb [18, NSLOT] f32, raster pixel cols ----
        off_sb = coords.tile([18, NSLOT], F32)
        nc.vector.memset(off_sb[:, NPIX:NSLOT], 0.0)
        with tc.tile_pool(name="po", bufs=2, space="PSUM") as po:
            for ns in range(4):
                ps_o = po.tile([18, 392], F32)
                for kc in range(18):
                    k, ch = divmod(kc, 2)
                    ky, kx = divmod(k, 3)
                    rhs = _ap(
                        xcf[:, :, :],
                        ch * 1740 + (ns * 7 + ky) * 58 + kx,
                        [[58, 7], [1, 56]],
                    )
                    nc.tensor.matmul(
                        ps_o[:, :],
                        woff[:, ch, k, :],
                        rhs,
                        start=(kc == 0),
                        stop=(kc == 17),
                    )
                nc.vector.tensor_scalar(
                    out=off_sb[:, ns * 392 : (ns + 1) * 392],
                    in0=ps_o[:, :],
                    scalar1=boff[:, 0:1],
                    scalar2=None,
                    op0=AL.add,
                )

        # ---- transpose offsets to pixel-on-partition [128, NBLK, 18] ----
        offT = coords.tile([128, NBLK, 18], F32)
        with tc.tile_pool(name="pot", bufs=1, space="PSUM") as pot:
            ps_t = pot.tile([128, NBLK, 18], F32)
            for bb in range(NBLK):
                nc.tensor.transpose(
                    ps_t[:, bb, :],
                    off_sb[:18, bb * BLK : (bb + 1) * BLK],
                    ident_f[:18, :18],
                )
            nc.vector.tensor_copy(out=offT[:, :, :], in_=ps_t[:, :, :])

        # ---- coordinate + weight math (fp32 [128, NBLK, 9] planes) ----
        _fc = [0]

        def floor_fix(dst_f, src, shape):
            """dst_f = floor(src) for src >= 0 (i32 round-to-nearest + fixup)."""
            _fc[0] += 1
            sl = (slice(None),) * len(shape)
            ci = coords.tile(shape, I32, name=f"ci{_fc[0]}")
            nc.vector.tensor_copy(out=ci[sl], in_=src[sl])
            nc.vector.tensor_copy(out=dst_f[sl], in_=ci[sl])
            gt = coords.tile(shape, F32, name=f"gt{_fc[0]}")
            nc.vector.tensor_tensor(
                out=gt[sl], in0=dst_f[sl], in1=src[sl], op=AL.is_gt
            )
            nc.vector.tensor_tensor(
                out=dst_f[sl], in0=dst_f[sl], in1=gt[sl], op=AL.subtract
            )

        # p = 128*bb + s; r = p//56; j = p%56
        p_i = coords.tile([128, NBLK], I32)
        nc.gpsimd.iota(p_i[:, :], pattern=[[BLK, NBLK]], base=0, channel_multiplier=1)
        p_f = coords.tile([128, NBLK], F32)
        nc.vector.tensor_copy(out=p_f[:, :], in_=p_i[:, :])
        t56 = coords.tile([128, NBLK], F32)
        nc.vector.tensor_scalar(
            out=t56[:, :], in0=p_f[:, :], scalar1=0.5, scalar2=1.0 / 56.0,
            op0=AL.add, op1=AL.mult,
        )
        r_f = coords.tile([128, NBLK], F32)
        floor_fix(r_f, t56, [128, NBLK])
        jx = coords.tile([128, NBLK], F32)
        nc.vector.scalar_tensor_tensor(
            out=jx[:, :], in0=r_f[:, :], scalar=-56.0, in1=p_f[:, :],
            op0=AL.mult, op1=AL.add,
        )

        kyM_i = coords.tile([128, KK], I32)
        nc.gpsimd.iota(
            kyM_i[:, :], pattern=[[1, 3], [0, 3]], base=MARG - 1, channel_multiplier=0
        )
        kyM = coords.tile([128, KK], F32)
        nc.vector.tensor_copy(out=kyM[:, :], in_=kyM_i[:, :])
        kxM_i = coords.tile([128, KK], I32)
        nc.gpsimd.iota(
            kxM_i[:, :], pattern=[[0, 3], [1, 3]], base=MARG - 1, channel_multiplier=0
        )
        kxM = coords.tile([128, KK], F32)
        nc.vector.tensor_copy(out=kxM[:, :], in_=kxM_i[:, :])

        dy = _ap(offT[:], 0, [[18, NBLK], [2, KK]])
        dx = _ap(offT[:], 1, [[18, NBLK], [2, KK]])
        r_b = _ap(r_f[:], 0, [[1, NBLK], [0, KK]])
        jx_b = _ap(jx[:], 0, [[1, NBLK], [0, KK]])
        kyM_b = _ap(kyM[:], 0, [[0, NBLK], [1, KK]])
        kxM_b = _ap(kxM[:], 0, [[0, NBLK], [1, KK]])

        P3 = [128, NBLK, KK]
        pym = coords.tile(P3, F32)
        pxm = coords.tile(P3, F32)
        # first add walks (k outer, bb inner) so the broadcast operand has a
        # stride-1 innermost dim (a 0-stride innermost dim is ~30x slower)
        dy_kb = _ap(offT[:], 0, [[2, KK], [18, NBLK]])
        dx_kb = _ap(offT[:], 1, [[2, KK], [18, NBLK]])
        r_kb = _ap(r_f[:], 0, [[0, KK], [1, NBLK]])
        jx_kb = _ap(jx[:], 0, [[0, KK], [1, NBLK]])
        pym_kb = _ap(pym[:], 0, [[1, KK], [KK, NBLK]])
        pxm_kb = _ap(pxm[:], 0, [[1, KK], [KK, NBLK]])
        nc.vector.tensor_tensor(out=pym_kb, in0=dy_kb, in1=r_kb, op=AL.add)
        nc.vector.tensor_tensor(out=pym[:, :, :], in0=pym[:, :, :], in1=kyM_b, op=AL.add)
        nc.vector.tensor_tensor(out=pxm_kb, in0=dx_kb, in1=jx_kb, op=AL.add)
        nc.vector.tensor_tensor(out=pxm[:, :, :], in0=pxm[:, :, :], in1=kxM_b, op=AL.add)

        y0 = coords.tile(P3, F32)
        x0 = coords.tile(P3, F32)
        floor_fix(y0, pym, P3)
        floor_fix(x0, pxm, P3)
        ty = coords.tile(P3, F32)
        tx = coords.tile(P3, F32)
        nc.vector.tensor_tensor(
            out=ty[:, :, :], in0=pym[:, :, :], in1=y0[:, :, :], op=AL.subtract
        )
        nc.vector.tensor_tensor(
            out=tx[:, :, :], in0=pxm[:, :, :], in1=x0[:, :, :], op=AL.subtract
        )

        # quad-table row index — idx chain FIRST (it gates the gathers; the
        # alpha/diag work below can overlap the fold DMAs)
        idxf = coords.tile(P3, F32)
        nc.vector.scalar_tensor_tensor(
            out=idxf[:, :, :], in0=y0[:, :, :], scalar=float(WQ), in1=x0[:, :, :],
            op0=AL.mult, op1=AL.add,
        )

        # ---- fold indices into SWDGE wrapped layout ----
        # idxw[16m+q, bb, k*8+t] = idx[s=16t+q, bb, k]
        # route: PE transpose -> idxT16 (wrap-permuted) -> DRAM -> one strided
        # DMA back into group 0 -> replicate to groups 1..7.
        idxd = nc.dram_tensor("idxd", [117 * 128], I16, kind="Internal")
        idxT16 = coords.tile([117, 16, 8], I16)
        with tc.tile_pool(name="pidx", bufs=1, space="PSUM") as pidx:
            ps_i = pidx.tile([117, 128], F32)
            nc.tensor.transpose(
                ps_i[:, :], _ap(idxf[:, :, :], 0, [[1, 117]]), ident_f[:, :]
            )
            # permute columns to wrap order: dst col q*8+t <- pixel 16t+q
            nc.vector.tensor_copy(
                out=idxT16[:, :, :], in_=_ap(ps_i[:, :], 0, [[1, 16], [16, 8]])
            )
        nc.sync.dma_start(
            out=bass.AP(tensor=idxd, offset=0, ap=[[128, 117], [1, 128]]),
            in_=idxT16[:, :, :],
        )

        idxw = coords.tile([128, NBLK, 72], I16)
        ppw = idxw[:, :, :].ap[0][0]
        dma_engines = [nc.sync, nc.scalar]
        # one DMA: dst walk (q, bb, k, t); src linear (bb*9+k)*128 + q*8 + t
        nc.scalar.dma_start(
            out=bass.AP(
                tensor=idxw.tensor,
                offset=idxw.offset,
                ap=[[ppw, 16], [72, NBLK], [8, KK], [1, 8]],
            ),
            in_=bass.AP(
                tensor=idxd,
                offset=0,
                ap=[[8, 16], [KK * 128, NBLK], [128, KK], [1, 8]],
            ),
        )
        rep = NBLK * 72
        # replicate group 0 -> groups 1..7 (independent copies, spread queues;
        # low groups first — queue q's core pair only reads groups <= 2q+1)
        for i, m in enumerate((1, 2, 3, 4, 5, 6, 7)):
            src = bass.AP(tensor=idxw.tensor, offset=idxw.offset, ap=[[ppw, 16], [1, rep]])
            dst = bass.AP(
                tensor=idxw.tensor,
                offset=idxw.offset + 16 * m * ppw,
                ap=[[ppw, 16], [1, rep]],
            )
            dma_engines[i % 2].dma_start(out=dst, in_=src)

        # bilinear corner weights: q order (a,b,c,d) matches quad packing
        u = coords.tile(P3, F32)  # 1 - tx
        v = coords.tile(P3, F32)  # 1 - ty
        nc.vector.tensor_scalar(
            out=u[:, :, :], in0=tx[:, :, :], scalar1=-1.0, scalar2=1.0,
            op0=AL.mult, op1=AL.add,
        )
        nc.vector.tensor_scalar(
            out=v[:, :, :], in0=ty[:, :, :], scalar1=-1.0, scalar2=1.0,
            op0=AL.mult, op1=AL.add,
        )
        # [128, 4 q, NBLK, KK] f32 products (contiguous writes), then one
        # reorder-copy to bf16 [128, NBLK, 36] with kq = q*9+k flattened.
        alphas = coords.tile([128, 4, NBLK, KK], F32)
        for q, (fy, fx_) in enumerate(((v, u), (v, tx), (ty, u), (ty, tx))):
            nc.vector.tensor_tensor(
                out=alphas[:, q, :, :],
                in0=fy[:, :, :],
                in1=fx_[:, :, :],
                op=AL.mult,
            )
        ab16 = coords.tile([128, NBLK, 4, KK], BF16)
        nc.vector.tensor_copy(
            out=ab16[:, :, :, :],
            in_=_ap(
                alphas[:, :, :, :], 0,
                [[KK, NBLK], [NBLK * KK, 4], [1, KK]],
            ),
        )
        # duplicated-pair copy: per-block diag builds read alpha with a
        # stride-1 innermost [.,2] dim, keeping them eligible for DVE 2x mode
        adup = coords.tile([128, NBLK * 36, 2], BF16)
        nc.vector.tensor_copy(
            out=adup[:, :, :],
            in_=_ap(ab16[:, :, :, :], 0, [[1, NBLK * 36], [0, 2]]),
        )

        # ---- per-block: gather -> diag -> scaled transposes -> main conv ----
        # main matmul batches PAIRS of blocks (256-col streams amortize the
        # wm weight loads); NBLK=13 so the last "pair" is a single block.
        with (
            tc.tile_pool(name="gp", bufs=8) as gp,
            tc.tile_pool(name="dp", bufs=3) as dp,
            tc.tile_pool(name="rp", bufs=2) as rp,
            tc.tile_pool(name="osb", bufs=3) as osb,
            tc.tile_pool(name="ptp", bufs=2, space="PSUM") as ptp,
            tc.tile_pool(name="oup", bufs=2, space="PSUM") as oup,
        ):
            rhs_t = None
            for bb in range(NBLK):
                half = bb % 2
                g = gp.tile([128, KK, 1024], F8E3)
                if bb < 2:
                    # pipeline fill: split the first blocks' gathers into
                    # 3-tap sub-gathers on separate queues so descriptor
                    # generation for block 0 takes ~3.3us instead of ~10us
                    for j in range(3):
                        nc.gpsimd.dma_gather(
                            out_ap=g[:, 3 * j : 3 * j + 3, :],
                            in_ap=xq_d[:, :],
                            idxs_ap=idxw[:, bb, 24 * j : 24 * j + 24],
                            num_idxs=NIDX // 3,
                            num_idxs_reg=NIDX // 3,
                            elem_size=1024,
                            single_packet=False,
                            queue_num=(bb * 3 + j) % 4,
                        )
                else:
                    nc.gpsimd.dma_gather(
                        out_ap=g[:, :, :],
                        in_ap=xq_d[:, :],
                        idxs_ap=idxw[:, bb, :],
                        num_idxs=NIDX,
                        num_idxs_reg=NIDX,
                        elem_size=1024,
                        single_packet=False,
                        queue_num=bb % 4,
                    )
                # diag[p, kq, j] = ident[p, j] * alpha[p, kq]: kq-major so the
                # matmul rhs columns stay contiguous (strided columns are ~4x
                # slower on the PE rhs stream); duplicated-pair APs keep all
                # innermost strides at 1 -> DVE 2x mode (2.8us vs 5.2us)
                diag = dp.tile([128, 36, 128], BF16)
                nc.vector.tensor_tensor(
                    out=_ap(diag[:, :, :], 0, [[128, 36], [2, 64], [1, 2]]),
                    in0=_ap(ident_b[:, :], 0, [[0, 36], [2, 64], [1, 2]]),
                    in1=_ap(adup[:, :, :], bb * 72, [[2, 36], [0, 64], [1, 2]]),
                    op=AL.mult,
                )
                if half == 0:
                    rhs_t = rp.tile([128, KK, 2, 2, 128], BF16, tag="rhs", name="rhs")
                for grp in range(3):
                    pt = ptp.tile([128, 3, 2, 128], F32, tag="pt", name="pt")
                    for kk in range(3):
                        k = grp * 3 + kk
                        for ch in range(2):
                            for q in range(4):
                                nc.tensor.matmul(
                                    pt[:, kk, ch, :],
                                    _ap(g[:, :, :], k * 1024 + q * 256 + ch * 128, [[1, 128]]),
                                    diag[:, q * KK + k, :],
                                    start=(q == 0),
                                    stop=(q == 3),
                                )
                    nc.scalar.copy(
                        out=rhs_t[:, grp * 3 : (grp + 1) * 3, :, half, :],
                        in_=pt[:, :, :, :],
                    )
                if half == 1 or bb == NBLK - 1:
                    ncols = (half + 1) * 128
                    b0 = bb - half
                    # each oh group gets its own 2KB PSUM bank (start=True
                    # zeroes the whole bank; groups must not share one)
                    outp = oup.tile([128, 2, 512], F32, tag="outp", name="outp")
                    for kc in range(18):
                        k, ch = divmod(kc, 2)
                        for oh in range(2):
                            nc.tensor.matmul(
                                outp[:, oh, 0:ncols],
                                wm[:, k, ch, oh, :],
                                _ap(rhs_t[:, :, :, :, :], (k * 2 + ch) * 256, [[1, ncols]]),
                                start=(kc == 0),
                                stop=(kc == 17),
                            )
                    o_t = osb.tile([128, 2, 256], F32, tag="ot", name="ot")
                    # DVE, not Act: Act's pt copies gate each block's main
                    # matmul; the output copy rides on DVE slack instead
                    nc.vector.tensor_copy(
                        out=o_t[:, :, 0:ncols], in_=outp[:, :, 0:ncols]
                    )
                    nc.sync.dma_start(
                        out=out_d[:, :, b0 * BLK : b0 * BLK + ncols],
                        in_=_ap(o_t[:, :, :], 0, [[256, 2], [1, ncols]]),
                    )

    nc.compile()
    return nc


def prep_inputs(x, w_off, b_off, w):
    """Host-side slab/layout prep. Returns list of 8 per-core input dicts."""
    x = np.asarray(x, dtype=np.float32)
    w_off = np.asarray(w_off, dtype=np.float32)
    b_off = np.asarray(b_off, dtype=np.float32)
    w = np.asarray(w, dtype=np.float32)

    woff_arr = np.ascontiguousarray(
        w_off.reshape(18, 2, 128, KK).transpose(2, 1, 3, 0)
    ).astype(ml_dtypes.bfloat16)  # [128 cl, 2 ch, 9 k, 18 o]
    boff_arr = np.ascontiguousarray(b_off.reshape(18, 1))
    wm_arr = np.ascontiguousarray(
        w.reshape(2, 128, 2, 128, KK).transpose(3, 4, 2, 0, 1)
    ).astype(ml_dtypes.bfloat16)  # [128 cl, 9 k, 2 ch, 2 ot, 128 ol]

    in_maps = []
    for core in range(8):
        b, half = divmod(core, 2)
        r0 = half * NROWS
        xb = x[b]  # [256, 56, 56]

        xp58 = np.zeros((CIN, 58, 58), np.float32)
        xp58[:, 1:57, 1:57] = xb
        xcf = np.ascontiguousarray(
            xp58[:, r0 : r0 + 30, :].reshape(2, 128, 30 * 58).transpose(1, 0, 2)
        ).astype(ml_dtypes.bfloat16)

        xp = np.zeros((HQ + 1, WQ + 1, CIN), np.float32)
        ylo = max(0, r0 - MARG)
        yhi = min(H, r0 + HQ + 1 - MARG)
        xhwc = xb.transpose(1, 2, 0)
        xp[ylo - (r0 - MARG) : yhi - (r0 - MARG), MARG : MARG + W, :] = xhwc[ylo:yhi]
        quad = np.stack(
            [xp[:-1, :-1], xp[:-1, 1:], xp[1:, :-1], xp[1:, 1:]], axis=2
        )  # [72, 72, 4, 256]
        xq = np.ascontiguousarray(quad.reshape(NQ, 4 * CIN)).astype(
            ml_dtypes.float8_e3m4
        )

        in_maps.append(
            {
                "xcf": xcf,
                "xq": xq,
                "woff": woff_arr,
                "boff": boff_arr,
                "wm": wm_arr,
            }
        )
    return in_maps


def unshard_output(results):
    """results: list of 8 per-core out arrays [128, 2, NSLOT] -> [B,COUT,H,W]."""
    out = np.zeros((B, COUT, H, W), np.float32)
    for core in range(8):
        b, half = divmod(core, 2)
        r0 = half * NROWS
        oc = results[core]  # [128 ol, 2 oh, NSLOT]
        oc = oc.transpose(1, 0, 2).reshape(COUT, NSLOT)[:, :NPIX]
        out[b, :, r0 : r0 + NROWS, :] = oc.reshape(COUT, NROWS, W)
    return out


def kernel(**inputs):
    nc = _CACHE.get("nc")
    if nc is None:
        nc = build_nc()
        _CACHE["nc"] = nc
    in_maps = prep_inputs(
        inputs["x"], inputs["w_off"], inputs["b_off"], inputs["w"]
    )
    res = run_bass_kernel_spmd(nc, in_maps, core_ids=list(range(8)))
    return unshard_output([r["out"] for r in res.results])
